# revision 13
# baseline (speedup 1.0000x reference)
"""Trainium2 Bass kernel for nn_CruxMiniCircuit (gnn_message_passing).

Reference semantics: B independent rows; each row is a circuit of N nodes
(literal nodes hold a fixed one-hot distribution over 10 ints, op nodes
combine left/right child distributions through a per-op bilinear table
followed by softmax).  The reference runs 10 synchronous passes over all
nodes and returns only the root (node 0) logits per row.

Device strategy (v3):
  * Only node 0's dependency cone matters, and a node's value stabilizes at
    pass h(n) = 1 + max(h(left), h(right)) (h=0 for literals).  Each cone
    node is computed ONCE at its stability height (memoization); nodes on
    cycles (h > 10) are computed per needed pass.  ~2.5x fewer updates than
    per-pass worklists.
  * Pass 1 is input-independent: value_1 is softmax(op_table[o,a,b,:]) if
    both children are literals (a 300-entry table) else the uniform vector
    (a zero operand kills the bilinear form).  The device softmaxes shipped
    logits once; everything else is a constant column.
  * The value buffer is replicated in 4 SBUF partition blocks at bases
    {0,32,64,96} so one `ap_gather` (channels=128) runs on 4 Q7 cores in
    parallel (ap_gather costs ~27ns/index/core and dominates otherwise).
    Replication is free: the bilinear-result matmul uses a (100,106) lhsT
    with W_o copies at the 4 block bases, making the whole softmax tail
    (106, PT)-shaped -- lockstep engines charge by free size only -- and
    the final multiply writes all 4 blocks in one instruction.
  * Zero per-pass DMAs: the gather output feeds the matmuls directly
    (matmul lhsT/rhs base partitions must match in {0,32,64,96}, so the
    l/r column halves sit in blocks 0/32 and 64/96), and the softmax
    multiply writes straight back into the value buffer.
  * Root logits are staged in SBUF per pass and shipped once at the end.

Sharding: pure data parallel over the batch dim (rows are LPT-balanced
across the 8 cores by update count).  No collectives needed.
"""

import sys
from contextlib import ExitStack

import numpy as np

sys.path.insert(0, "/opt/trn_rl_repo")

import concourse.bass as bass
import concourse.tile as tile
from concourse import bacc, mybir
from concourse.bass_utils import run_bass_kernel_spmd

B, N = 2048, 1023
NI, NO, NP = 10, 3, 10
NCORES = 8
HINF = NP + 1

ZSLOT = NI          # value-buffer col 10 = zero vector
USLOT = NI + 1      # col 11 = uniform 0.1 vector
TBASE = NI + 2      # cols 12..311 = pass-1 table softmax(W[o,a,b,:])
T1 = NO * NI * NI   # 300
BASE2 = TBASE + T1  # 312

NB = 6                 # gather blocks: partitions 96-127 (PE quadrant 3)
                       # corrupt matmul reads, so only Q7 cores 0-5 carry data
PB = tuple(range(0, 16 * NB, 16))
MW = 16 * (NB - 1) + NI  # widened partition dim: NB copies at 16-stride

# weight-pack column layout (pack dram/sbuf tensor, 128 x CW)
# The 8 front-matmul selector matrices are (128, 100) with the replication
# pattern at one block's rows and zeros elsewhere: the matmul contracts the
# full 128-partition gather output at base partition 0 (offset-base matmul
# operands crash the device), so zero rows null the other blocks' data.
PK_W4 = 0              # rows 0..99: w4_o cols [o*MW + 16g + k] = W[o,i,j,k]
PK_SEL = NO * MW       # NB x 100: sel[g] rows 16g..16g+9 = repl (g<NB/2) / reprm
PK_ONES10 = PK_SEL + 100 * NB  # rows 0..9: ones (10 x 1)
PK_ONES4 = PK_ONES10 + 1  # row 0: cols [16g+k] = 1  (1 x MW)
PK_CONST = PK_ONES4 + MW  # rows {pb..pb+15}: const block + host-softmaxed
                          # pass-1 table (x BASE2)
CW = PK_CONST + BASE2

TRACE = False
LAST_RESULTS = None


def _pad(x, m):
    return int(-(-x // m) * m)


def _plan(cats, ops, lits, left, right, mask):
    """Integer-only host preprocessing: heights, canonical worklists,
    core assignment, op-grouped columns, gather indices, output map."""
    left = np.clip(left.astype(np.int64), 0, N - 1)
    right = np.clip(right.astype(np.int64), 0, N - 1)
    opsc = np.clip(ops.astype(np.int64), 0, NO - 1)
    litsc = np.clip(lits.astype(np.int64), 0, NI - 1)
    m = mask.astype(bool)
    is_lit = (cats == 0) & m
    is_op = (cats == 1) & m

    # stability heights
    h = np.where(is_op, HINF, 0).astype(np.int64)
    for _ in range(NP):
        hl = np.take_along_axis(h, left, axis=1)
        hr = np.take_along_axis(h, right, axis=1)
        h = np.where(is_op, np.minimum(1 + np.maximum(hl, hr), HINF), 0)

    # canonical request sets W[c] = unique (r, n) needed at pass c (2..NP)
    r_op_root = np.nonzero(is_op[:, 0])[0].astype(np.int64)
    c_root = np.minimum(NP, h[r_op_root, 0])
    frontiers = {c: [] for c in range(2, NP + 1)}
    for c in range(2, NP + 1):
        sel = c_root == c
        frontiers[c].append((r_op_root[sel], np.zeros(int(sel.sum()), np.int64)))
    W = {}
    for c in range(NP, 1, -1):
        if frontiers[c]:
            rr = np.concatenate([f[0] for f in frontiers[c]])
            nn = np.concatenate([f[1] for f in frontiers[c]])
        else:
            rr = np.zeros(0, np.int64)
            nn = np.zeros(0, np.int64)
        kk = rr * N + nn
        _, uidx = np.unique(kk, return_index=True)
        rr, nn = rr[uidx], nn[uidx]
        W[c] = (rr, nn)
        for ch in (left[rr, nn], right[rr, nn]):
            cop = is_op[rr, ch]
            rc, nc_ = rr[cop], ch[cop]
            cc = np.minimum(c - 1, h[rc, nc_])
            for c2 in range(2, c):
                sel = cc == c2
                if sel.any():
                    frontiers[c2].append((rc[sel], nc_[sel]))

    # core assignment: greedy minimizing growth of per-(pass,op) cross-core
    # maxima (those maxima set the padded column counts every core pays for)
    D = (NP - 1) * NO
    rowvec = np.zeros((B, D), np.int64)
    for c in range(2, NP + 1):
        rr, nn = W[c]
        np.add.at(rowvec, (rr, (c - 2) * NO + opsc[rr, nn]), 1)
    wrow = rowvec.sum(1)
    core_of = np.zeros(B, np.int64)
    order = np.argsort(-wrow, kind="stable")
    loadv = np.zeros((NCORES, D), np.int64)
    tload = np.zeros(NCORES, np.int64)
    mx = np.zeros(D, np.int64)
    for r in order:
        if wrow[r] == 0:
            continue
        inc = np.maximum(loadv + rowvec[r] - mx, 0).sum(1)
        g = int(np.lexsort((tload, inc))[0])
        core_of[r] = g
        loadv[g] += rowvec[r]
        tload[g] += wrow[r]
        mx = np.maximum(mx, loadv[g])

    # per-pass op-grouped column assignment (group sizes common across cores)
    PTs = {}            # padded columns per pass (multiple of 32)
    goff = {}           # per pass: [off0, off1, off2, PT]
    colmap = {}         # per pass: (rr, nn, core, col)
    slot_stable = np.full((B, N), -1, np.int64)
    base = {}
    nextbase = BASE2
    for c in range(2, NP + 1):
        rr, nn = W[c]
        core = core_of[rr]
        og = opsc[rr, nn]
        cnt = np.zeros((NCORES, NO), np.int64)
        np.add.at(cnt, (core, og), 1)
        G = [_pad(int(cnt[:, o].max()), 4) if cnt[:, o].max() else 0 for o in range(NO)]
        PT = _pad(max(sum(G), 48), 24)
        G[NO - 1] += PT - sum(G)
        off = [0, G[0], G[0] + G[1], PT]
        key = core * NO + og
        sorder = np.argsort(key, kind="stable")
        ks = key[sorder]
        rank = np.arange(len(ks), dtype=np.int64)
        if len(ks):
            first = np.r_[True, ks[1:] != ks[:-1]]
            seg = np.nonzero(first)[0]
            rank = rank - seg[np.cumsum(first) - 1]
        ranks = np.empty(len(ks), np.int64)
        ranks[sorder] = rank
        col = np.array(off, np.int64)[og] if len(og) else np.zeros(0, np.int64)
        col = col + ranks
        PTs[c] = PT
        goff[c] = off
        colmap[c] = (rr, nn, core, col)
        base[c] = nextbase
        nextbase += PT
    S = nextbase

    # slot maps + gather indices (4 blocks: l cols halves -> blocks 0,2;
    # r cols halves -> blocks 4,6; odd blocks junk)
    slot_prev = np.full((B, N), -1, np.int64)
    idx_parts = []
    Ftot = 0
    for c in range(2, NP + 1):
        rr, nn, core, col = colmap[c]
        PT = PTs[c]
        lcol = np.full((NCORES, PT), ZSLOT, np.int64)
        rcol = np.full((NCORES, PT), ZSLOT, np.int64)
        for side, dst in ((left, lcol), (right, rcol)):
            ch = side[rr, nn]
            chlit = is_lit[rr, ch]
            hc = np.minimum(c - 1, h[rr, ch])
            s = np.where(chlit, litsc[rr, ch], ZSLOT)
            sel1 = (~chlit) & (hc == 1)
            if sel1.any():
                rc, nc_ = rr[sel1], ch[sel1]
                gl, gr = left[rc, nc_], right[rc, nc_]
                bothlit = is_lit[rc, gl] & is_lit[rc, gr]
                tslot = TBASE + opsc[rc, nc_] * 100 + 10 * litsc[rc, gl] + litsc[rc, gr]
                s[sel1] = np.where(bothlit, tslot, USLOT)
            sel2 = (~chlit) & (hc >= 2)
            if sel2.any():
                rc, nc_ = rr[sel2], ch[sel2]
                stab = h[rc, nc_] <= c - 1
                sv = np.where(stab, slot_stable[rc, nc_], slot_prev[rc, nc_])
                assert (sv >= 0).all(), "unresolved child slot"
                s[sel2] = sv
            dst[core, col] = s
        slot_prev = np.full((B, N), -1, np.int64)
        stab = h[rr, nn] == c
        slot_stable[rr[stab], nn[stab]] = base[c] + col[stab]
        slot_prev[rr[~stab], nn[~stab]] = base[c] + col[~stab]
        # per-block index arrays: NB blocks, block k takes l cols
        # [kC:(k+1)C], block k+NB/2 takes r cols [kC:(k+1)C], C = 2PT/NB
        H = NB // 2
        C = PT // H
        ni = _pad(C, 16)
        F = ni // 16
        idxw = np.full((NCORES, 8, 16, F), ZSLOT, np.int64)
        for k in range(H):
            for blk, colset in ((k, lcol[:, k * C:(k + 1) * C]),
                                (k + H, rcol[:, k * C:(k + 1) * C])):
                tmp = np.full((NCORES, F * 16), ZSLOT, np.int64)
                tmp[:, 0:C] = colset
                idxw[:, blk] = tmp.reshape(NCORES, F, 16).transpose(0, 2, 1)
        idx_parts.append(idxw.reshape(NCORES, 128, F).astype(np.int16))
        Ftot += F
    idx_full = np.concatenate(idx_parts, axis=2) if idx_parts else np.zeros((NCORES, 128, 0), np.int16)

    # output map
    offz = {}
    z = 0
    for c in range(2, NP + 1):
        offz[c] = z
        z += PTs[c]
    ZW = z
    root_entries = []  # (r, core, zcol)
    colof = {}
    for c in range(2, NP + 1):
        rr, nn, core, col = colmap[c]
        sel = nn == 0
        for r_, co_, cl_ in zip(rr[sel], core[sel], col[sel]):
            colof[(r_, c)] = (co_, offz[c] + cl_)
    for r_, c_ in zip(r_op_root, c_root):
        if c_ >= 2:
            co_, zc_ = colof[(r_, min(NP, c_))]
            root_entries.append((r_, co_, zc_))

    return dict(
        PTs=PTs, goff=goff, base=base, S=S, idx=idx_full, Ftot=Ftot,
        ZW=ZW, offz=offz, root_entries=root_entries,
        r_op_root=r_op_root, c_root=c_root,
        opsc=opsc, litsc=litsc, is_lit=is_lit, left=left, right=right,
    )


def _build_nc(S, PTs, goff, base, Ftot, ZW):
    f32 = mybir.dt.float32
    nc = bacc.Bacc(None)
    pack = nc.dram_tensor("pack", [128, CW], f32, kind="ExternalInput")
    idx_in = nc.dram_tensor("idx", [128, max(Ftot, 2)], mybir.dt.int16, kind="ExternalInput")
    outz = nc.dram_tensor("outz", [NI, ZW], f32, kind="ExternalOutput")

    with ExitStack() as ctx:
        tc = ctx.enter_context(tile.TileContext(nc))
        singles = ctx.enter_context(tc.tile_pool(name="singles", bufs=1))
        work = ctx.enter_context(tc.tile_pool(name="work", bufs=2))
        psum = ctx.enter_context(tc.tile_pool(name="psum", bufs=1, space="PSUM"))

        buf = singles.tile([128, S], f32)
        nc.sync.dma_start(out=buf[:, 0:BASE2], in_=pack[:, PK_CONST:PK_CONST + BASE2])
        idx_sb = singles.tile([128, max(Ftot, 2)], mybir.dt.int16)
        nc.sync.dma_start(out=idx_sb[:, :], in_=idx_in[:, :])
        pack_sb = singles.tile([128, CW], f32)
        nc.sync.dma_start(out=pack_sb[:, :], in_=pack[:, :])
        nc.vector.memset(buf[:, BASE2:S], 0.0)
        outz_sb = singles.tile([NI, ZW], f32)

        # dummy gather: preload the GPSIMD ucode library during startup DMAs
        dmy_idx = singles.tile([128, 2], mybir.dt.int16)
        nc.vector.memset(dmy_idx[:, :], ZSLOT)
        dmy_out = singles.tile([128, 32], f32)
        nc.gpsimd.ap_gather(
            out_ap=dmy_out[:, :], in_ap=buf[:, 0:TBASE], idxs_ap=dmy_idx[:, :],
            channels=128, num_elems=TBASE, d=1, num_idxs=32,
        )

        def tail(z_ap, dst_ap, PT):
            # z_ap is (MW, PT) with logit copies at partition bases PB
            e = work.tile([MW, PT], f32, tag="e")
            nc.scalar.activation(e[:, :], z_ap, mybir.ActivationFunctionType.Exp)
            ps_z3 = psum.tile([1, PT], f32, tag="z3")
            nc.tensor.matmul(ps_z3[:, :], pack_sb[0:NI, PK_ONES10:PK_ONES10 + 1],
                             e[0:NI, :], start=True, stop=True)
            rz = work.tile([1, PT], f32, tag="rz")
            nc.vector.reciprocal_approx_fast(rz[:, :], ps_z3[:, :])
            ps_rz = psum.tile([MW, PT], f32, tag="rz2")
            nc.tensor.matmul(ps_rz[:, :], pack_sb[0:1, PK_ONES4:PK_ONES4 + MW],
                             rz[:, :], start=True, stop=True)
            nc.vector.tensor_mul(dst_ap, e[:, :], ps_rz[:, :])

        foff = 0
        zoff = 0
        for c in range(2, NP + 1):
            PT = PTs[c]
            H = NB // 2
            C = PT // H
            ni = _pad(C, 16)
            F = ni // 16
            off = goff[c]
            lr = work.tile([128, ni], f32, tag="lr")
            nc.gpsimd.ap_gather(
                out_ap=lr[:, :], in_ap=buf[:, :],
                idxs_ap=idx_sb[:, foff:foff + F],
                channels=128, num_elems=S, d=1, num_idxs=ni,
            )
            foff += F
            ps_l = psum.tile([100, PT], f32, tag="ps_l")
            for k in range(H):
                nc.tensor.matmul(ps_l[:, k * C:(k + 1) * C],
                                 pack_sb[:, PK_SEL + k * 100:PK_SEL + (k + 1) * 100],
                                 lr[:, 0:C], start=True, stop=True)
            ps_r = psum.tile([100, PT], f32, tag="ps_r")
            for k in range(H):
                nc.tensor.matmul(ps_r[:, k * C:(k + 1) * C],
                                 pack_sb[:, PK_SEL + (k + H) * 100:PK_SEL + (k + H + 1) * 100],
                                 lr[:, 0:C], start=True, stop=True)
            lsb = work.tile([100, PT], f32, tag="lsb")
            nc.scalar.copy(lsb[:, :], ps_l[:, :])
            outer = work.tile([100, PT], f32, tag="outer")
            nc.vector.tensor_mul(outer[:, :], lsb[:, :], ps_r[:, :])
            ps_z = psum.tile([MW, PT], f32, tag="ps_z")
            for o in range(NO):
                a, b = off[o], off[o + 1]
                if a == b:
                    continue
                nc.tensor.matmul(ps_z[:, a:b], pack_sb[0:100, PK_W4 + o * MW:PK_W4 + (o + 1) * MW],
                                 outer[:, a:b], start=True, stop=True)
            if c < NP:
                tail(ps_z[:, :], buf[0:MW, base[c]:base[c] + PT], PT)
                # dependency-free junk matmuls fill the next gather's PE-idle
                # window to hold the HAM clock-gate at 8/8 (cold PE halves
                # matmul throughput); sized to ~50% of the window
                ni_next = _pad(PTs[c + 1] // (NB // 2), 16)
                nj = max(3, (ni_next * 27 + 400) // 380)
                ps_j = psum.tile([1, 4], f32, tag="junk")
                for _ in range(nj):
                    nc.tensor.matmul(ps_j[:, :], pack_sb[0:1, PK_ONES10:PK_ONES10 + 1],
                                     pack_sb[0:1, 0:4], start=True, stop=True)
            nc.scalar.copy(outz_sb[:, zoff:zoff + PT], ps_z[0:NI, :])
            zoff += PT
        nc.sync.dma_start(out=outz[:, :], in_=outz_sb[:, :])
    nc.finalize()
    return nc


def _make_pack(op_table):
    pack = np.zeros((128, CW), np.float32)
    w30 = op_table.transpose(1, 2, 0, 3).reshape(100, NO * NI)  # col o*10+k
    for o in range(NO):
        for pb in PB:
            pack[0:100, PK_W4 + o * MW + pb:PK_W4 + o * MW + pb + NI] = \
                w30[:, o * NI:(o + 1) * NI]
    repl = np.kron(np.eye(NI), np.ones((1, NI))).astype(np.float32)
    reprm = np.tile(np.eye(NI), (1, NI)).astype(np.float32)
    for g in range(NB):
        sel = repl if g < NB // 2 else reprm
        pack[16 * g:16 * g + NI, PK_SEL + g * 100:PK_SEL + (g + 1) * 100] = sel
    pack[0:NI, PK_ONES10] = 1.0
    for pb in PB:
        pack[0, PK_ONES4 + pb:PK_ONES4 + pb + NI] = 1.0
    consts = np.zeros((16, BASE2), np.float32)
    consts[0:NI, 0:NI] = np.eye(NI)
    consts[0:NI, USLOT] = 1.0 / NI
    e1 = np.exp(op_table.reshape(T1, NI).T)  # pass-1 table, softmaxed on host
    consts[0:NI, TBASE:BASE2] = e1 / e1.sum(0, keepdims=True)
    for pb in PB:
        pack[pb:pb + 16, PK_CONST:PK_CONST + BASE2] = consts
    return pack


def _emulate(plan, pack):
    """Numpy emulation of the exact device program (for validation).
    Emulates logical block 0/2 (l) and 4/6 (r) gathers from a single buffer
    copy since all 4 blocks hold identical data."""
    S, PTs, goff, base, ZW = plan["S"], plan["PTs"], plan["goff"], plan["base"], plan["ZW"]
    idx = plan["idx"].astype(np.int64)  # (NCORES, 128, Ftot)
    outz = np.zeros((NCORES, NI, ZW), np.float32)
    w30 = pack[0:100, PK_W4:PK_W4 + NI]  # dummy; real read below
    for core in range(NCORES):
        buf = np.zeros((16, S), np.float32)
        buf[:, 0:BASE2] = pack[0:16, PK_CONST:PK_CONST + BASE2]

        def tail(z, dst_sl):
            e = np.exp(z)
            zsum = e.sum(axis=0, keepdims=True)
            buf[0:NI, dst_sl] = e / zsum

        foff = 0
        zoff = 0
        for c in range(2, NP + 1):
            PT = PTs[c]
            H = NB // 2
            C = PT // H
            ni = C + (-C % 16)
            F = ni // 16
            iw = idx[core, :, foff:foff + F]  # (128, F)
            foff += F

            def cols_of(blk):
                return iw[16 * blk:16 * blk + 16].T.reshape(F * 16)[0:C]

            lcols = np.concatenate([cols_of(k) for k in range(H)])
            rcols = np.concatenate([cols_of(k + H) for k in range(H)])
            l, r = buf[0:NI][:, lcols], buf[0:NI][:, rcols]
            outer = np.einsum('iu,ju->iju', l, r).reshape(100, PT)
            z = np.zeros((NI, PT), np.float32)
            off = goff[c]
            for o in range(NO):
                a, b = off[o], off[o + 1]
                if a == b:
                    continue
                w_o = pack[0:100, PK_W4 + o * MW:PK_W4 + o * MW + NI]
                z[:, a:b] = w_o.T @ outer[:, a:b]
            outz[core][:, zoff:zoff + PT] = z
            zoff += PT
            if c < NP:
                tail(z, slice(base[c], base[c] + PT))
    return outz


def _assemble(plan, op_table, outz_per_core):
    out = np.zeros((B, NI), np.float32)
    litsc, is_lit = plan["litsc"], plan["is_lit"]
    opsc, left, right = plan["opsc"], plan["left"], plan["right"]
    lit_rows = np.nonzero(is_lit[:, 0])[0]
    out[lit_rows] = 10.0 * np.eye(NI, dtype=np.float32)[litsc[lit_rows, 0]]
    for r_, c_ in zip(plan["r_op_root"], plan["c_root"]):
        if c_ == 1:
            a = litsc[r_, left[r_, 0]]
            b = litsc[r_, right[r_, 0]]
            out[r_] = op_table[opsc[r_, 0], a, b]
    for r_, core_, zc_ in plan["root_entries"]:
        out[r_] = outz_per_core[core_][:, zc_]
    return out


def kernel(op_table, cats, ops, lits, left, right, mask, _emulate_only=False):
    global LAST_RESULTS
    op_table = np.asarray(op_table, np.float32)
    plan = _plan(np.asarray(cats), np.asarray(ops), np.asarray(lits),
                 np.asarray(left), np.asarray(right), np.asarray(mask))
    pack = _make_pack(op_table)
    assert plan["S"] <= 32000, plan["S"]
    assert max(plan["PTs"].values()) <= 512, plan["PTs"]

    if _emulate_only:
        outz = _emulate(plan, pack)
        return _assemble(plan, op_table, outz)

    nc = _build_nc(plan["S"], plan["PTs"], plan["goff"], plan["base"],
                   plan["Ftot"], plan["ZW"])

    in_maps = []
    for c in range(NCORES):
        in_maps.append({
            "pack": pack,
            "idx": np.ascontiguousarray(plan["idx"][c]) if plan["Ftot"] else np.zeros((128, 2), np.int16),
        })
    res = run_bass_kernel_spmd(nc, in_maps, list(range(NCORES)), trace=TRACE)
    LAST_RESULTS = res
    outz = [np.asarray(res.results[c]["outz"]) for c in range(NCORES)]
    return _assemble(plan, op_table, outz)
